# revision 10
# baseline (speedup 1.0000x reference)
"""CascadeGDCN (3-hop graph diffusion convolution) on 8 Trainium2 NeuronCores.

Strategy (v3):
  - Destination nodes sharded across the 8 cores (12544 rows each, padded to
    100352 total).  Edges partitioned by destination core.
  - Node features for BOTH chains packed per node: Z[v] = [X_out[v] | X_in[v]]
    as bf16, 128 wide = 256B rows (the dma_gather element granularity).
  - Gather-source node space split into two halves A (48 groups/core) and
    B (50 groups/core), renumbered so each half is contiguous across cores.
    One AllGather per half per hop; the A-half AllGather is triggered early
    (mid way through the second direction) so the next hop's A-chunk gathers
    never stall on the collective.
  - Per SpMM direction, two passes (chunk A then chunk B).  Edges bucketed
    into (128-dest-group x half) cells; signed int16 gather indices are
    relative to the half's midpoint.  Messages fetched with dma_gather;
    segment reduction on the TensorEngine as S^T @ M with a one-hot bf16 S
    built by one VectorEngine is_equal; edge values folded into the used
    64-wide half of the messages.  Per-cell slot counts are variable
    (max across cores).  Trailing padding of each gather call is marked with
    negative indices so the Q7 descriptor generator skips it.
  - Final stage: sum_term^T via PE transpose, z = Theta^T @ st_fm on PE,
    sigmoid on ScalarE, + H on VectorE, output written feature-major and
    transposed back on the host.
"""

import numpy as np
import ml_dtypes

D = 64
NCORES = 8
NUM_HOPS = 3
N_NODES = 100000
SHARD = 12544            # dest rows per core (98 groups of 128)
NODES_PAD = SHARD * NCORES   # 100352
GROUPS = SHARD // 128    # 98
GROUPS_A = 48            # groups per core in half A
SPLIT_LOCAL = GROUPS_A * 128      # 6144
NA = SPLIT_LOCAL * NCORES         # 49152 rows in half A
NB = NODES_PAD - NA               # 51200 rows in half B
MID = [NA // 2, NB // 2]          # signed-idx base offset per half
NCHUNKS = 2
GPB = 4                  # dest groups per block (per gather call)
SKIP_FINAL = False       # dev bisect flag
SKIP_SPMM = False        # dev bisect flag

BF16 = ml_dtypes.bfloat16


def _softmax(x):
    e = np.exp(x - x.max())
    return e / e.sum()


def _new_id(v):
    """Renumber original node id -> packed [A | B] gather-space id."""
    m = v // SHARD
    loc = v % SHARD
    return np.where(loc < SPLIT_LOCAL,
                    m * SPLIT_LOCAL + loc,
                    NA + m * (SHARD - SPLIT_LOCAL) + (loc - SPLIT_LOCAL))


def _blocks():
    out = []
    g = 0
    while g < GROUPS:
        out.append((g, min(GPB, GROUPS - g)))
        g += GPB
    return out


def _layout(caps):
    """Edge-stream layout from per-cell slot counts caps[g, c].

    Stream order: [chunk c][block][group_in_block cells consecutively]
    (pass-major so each pass's calls are contiguous in the stream).
    """
    blocks = _blocks()
    call_off = [[0] * NCHUNKS for _ in blocks]
    call_slots = [[0] * NCHUNKS for _ in blocks]
    cell_soff = np.zeros((GROUPS, NCHUNKS), dtype=np.int64)
    pos = 0
    for c in range(NCHUNKS):
        for bi, (g0, gc) in enumerate(blocks):
            call_off[bi][c] = pos
            n = 0
            for gl in range(gc):
                cell_soff[g0 + gl, c] = pos + n
                n += int(caps[g0 + gl, c])
            pos += n
            call_slots[bi][c] = n
    return blocks, call_off, call_slots, cell_soff, pos


def _direction_caps(dest, src, shard):
    """caps[g, c] = max over cores of ceil(cell_count/128), min 1."""
    core = dest // shard
    c_of = ((src % shard) >= SPLIT_LOCAL).astype(np.int64)
    caps = np.zeros((GROUPS, NCHUNKS), dtype=np.int64)
    for m in range(NCORES):
        sel = core == m
        d_loc = dest[sel] - m * shard
        g = d_loc >> 7
        counts = np.bincount(g * NCHUNKS + c_of[sel],
                             minlength=GROUPS * NCHUNKS)
        caps = np.maximum(caps, (counts.reshape(GROUPS, NCHUNKS) + 127) // 128)
    return np.maximum(caps, 1)


def _prep_direction(dest, src, val, shard, caps, lay):
    """Per-core gather/S tables for one SpMM direction."""
    blocks, call_off, call_slots, cell_soff, tot_slots = lay
    tot = tot_slots * 128
    core = dest // shard
    src_new = _new_id(src)
    c_of = ((src % shard) >= SPLIT_LOCAL).astype(np.int64)
    base = np.where(c_of == 0, MID[0], NA + MID[1])
    idx_all = src_new - base                      # signed, fits int16
    out = []
    for m in range(NCORES):
        sel = core == m
        d_loc = (dest[sel] - m * shard).astype(np.int64)
        idxv = idx_all[sel].astype(np.int64)
        v = val[sel].astype(np.float32)
        g = d_loc >> 7
        cell = g * NCHUNKS + c_of[sel]
        order = np.lexsort((idxv, cell))
        cell_s = cell[order]
        counts = np.bincount(cell_s, minlength=GROUPS * NCHUNKS)
        starts = np.zeros(GROUPS * NCHUNKS, dtype=np.int64)
        starts[1:] = np.cumsum(counts)[:-1]
        rank = np.arange(cell_s.size) - starts[cell_s]
        pos = cell_soff.reshape(-1)[cell_s] * 128 + rank

        idx_st = np.zeros(tot, dtype=np.int16)
        denc_st = np.full(tot, -1.0, dtype=np.float32)
        val_st = np.zeros(tot, dtype=np.float32)
        idx_st[pos] = idxv[order].astype(np.int16)
        denc_st[pos] = (d_loc[order] & 127).astype(np.float32)
        val_st[pos] = v[order]

        # record this core's last-real-edge position per call (for the
        # uniform trailing-trim computed across cores by the caller)
        counts2 = counts.reshape(GROUPS, NCHUNKS)
        last_real = np.zeros((len(blocks), NCHUNKS), dtype=np.int64)
        for bi, (g0, gc) in enumerate(blocks):
            for c in range(NCHUNKS):
                gl_last = g0 + gc - 1
                last_real[bi, c] = (cell_soff[gl_last, c] * 128
                                    + counts2[gl_last, c])

        out.append({"idx_st": idx_st, "denc_st": denc_st, "val_st": val_st,
                    "last_real": last_real})

    # uniform trailing-trim point per call (max across cores), so that every
    # core's Q7 trims to the same count and it can be baked into num_idxs_reg
    trims = np.zeros((len(blocks), NCHUNKS), dtype=np.int64)
    for bi, (g0, gc) in enumerate(blocks):
        for c in range(NCHUNKS):
            call_end = (call_off[bi][c] + call_slots[bi][c]) * 128
            t = max(o["last_real"][bi, c] for o in out)
            # every core's entry at t-1 must be >= 0 so its trim stops at t
            if t == 0 or any(o["idx_st"][t - 1] < 0 for o in out):
                t = call_end
            trims[bi, c] = t - call_off[bi][c] * 128   # call-relative count
            for o in out:
                o["idx_st"][t:call_end] = -1

    tables = []
    for o in out:
        idx_tbl = np.tile(
            np.ascontiguousarray(o["idx_st"].reshape(-1, 16).T), (8, 1))
        denc_tbl = np.ascontiguousarray(
            o["denc_st"].reshape(-1, 128).T.astype(BF16))
        val_tbl = np.ascontiguousarray(
            o["val_st"].reshape(-1, 128).T.astype(BF16))
        tables.append({"idx": idx_tbl, "denc": denc_tbl, "val": val_tbl})
    return tables, trims


def prep_host(H_l, edge_row, edge_col, edge_val, out_degree, in_degree,
              hop_attention, theta_out, theta_in, Theta):
    """Host-side preprocessing: per-core input maps + meta for the builder."""
    H = np.asarray(H_l, dtype=np.float32)
    er = np.asarray(edge_row, dtype=np.int64)
    ec = np.asarray(edge_col, dtype=np.int64)
    ev = np.asarray(edge_val, dtype=np.float32)
    od = np.asarray(out_degree, dtype=np.float32)
    idg = np.asarray(in_degree, dtype=np.float32)

    alpha = _softmax(np.asarray(hop_attention, dtype=np.float64))
    th_o = np.asarray(theta_out, dtype=np.float64)
    th_i = np.asarray(theta_in, dtype=np.float64)
    coef = [(float(alpha[k] * th_o[k]), float(alpha[k] * th_i[k]))
            for k in range(len(alpha))]

    # dir 0 ("out" chain): dest=row, src=col; dir 1: transposed
    caps0 = _direction_caps(er, ec, SHARD)
    caps1 = _direction_caps(ec, er, SHARD)
    lay0 = _layout(caps0)
    lay1 = _layout(caps1)
    t0, trims0 = _prep_direction(er, ec, ev, SHARD, caps0, lay0)
    t1, trims1 = _prep_direction(ec, er, ev, SHARD, caps1, lay1)

    # packed hop-0 features Z = [D_out H | D_in H] bf16, in renumbered order
    zfull = np.zeros((NODES_PAD, 2 * D), dtype=BF16)
    perm = _new_id(np.arange(NODES_PAD))
    zsrc = np.zeros((NODES_PAD, 2 * D), dtype=BF16)
    zsrc[:N_NODES, :D] = (np.maximum(od, 1e-8)[:, None] * H).astype(BF16)
    zsrc[:N_NODES, D:] = (np.maximum(idg, 1e-8)[:, None] * H).astype(BF16)
    zfull[perm] = zsrc
    z0a = np.ascontiguousarray(zfull[:NA])
    z0b = np.ascontiguousarray(zfull[NA:])

    hpad = np.zeros((NODES_PAD, D), dtype=np.float32)
    hpad[:N_NODES] = H
    ident = np.eye(128, dtype=np.float32)
    theta = np.ascontiguousarray(np.asarray(Theta, dtype=np.float32))

    in_maps = []
    for m in range(NCORES):
        in_maps.append({
            "z0a": z0a, "z0b": z0b,
            "hfm": np.ascontiguousarray(hpad[m * SHARD:(m + 1) * SHARD].T),
            "theta": theta,
            "ident": ident,
            "idx0": t0[m]["idx"], "denc0": t0[m]["denc"], "val0": t0[m]["val"],
            "idx1": t1[m]["idx"], "denc1": t1[m]["denc"], "val1": t1[m]["val"],
        })
    meta = {"coef": coef, "caps": [caps0, caps1], "lay": [lay0, lay1],
            "trims": [trims0, trims1]}
    return in_maps, meta


def build_program(tc, ins, outs, meta):
    """Emit the full SPMD program into TileContext tc."""
    import concourse.mybir as mybir

    nc = tc.nc
    f32 = mybir.dt.float32
    bf16 = mybir.dt.bfloat16
    i16 = mybir.dt.int16
    EQ, MUL, ADD = (mybir.AluOpType.is_equal, mybir.AluOpType.mult,
                    mybir.AluOpType.add)

    coef = meta["coef"]
    lays = meta["lay"]
    maxns = max(max(max(cs) for cs in lay[2]) for lay in lays)

    # internal DRAM: per-half bounce shards + ping-pong per-half Z buffers
    shp = [SPLIT_LOCAL, SHARD - SPLIT_LOCAL]
    nhalf = [NA, NB]
    bounce = [[nc.dram_tensor(f"bounce{h}_{p}", [shp[h], 2 * D], bf16,
                              kind="Internal") for p in range(2)]
              for h in range(2)]
    zbuf = [[nc.dram_tensor(f"zbuf{h}_{p}", [nhalf[h], 2 * D], bf16,
                            kind="Internal", addr_space="Shared")
             for p in range(2)] for h in range(2)]

    tabs = [
        (ins["idx0"], ins["denc0"], ins["val0"]),
        (ins["idx1"], ins["denc1"], ins["val1"]),
    ]
    rg = [list(range(NCORES))]

    with (
        tc.tile_pool(name="const", bufs=1) as cpool,
        tc.tile_pool(name="work", bufs=1) as wpool,
        tc.tile_pool(name="stream", bufs=4) as spool,
        tc.tile_pool(name="spool2", bufs=4) as spool2,
        tc.tile_pool(name="xn", bufs=2) as xpool,
        tc.tile_pool(name="xtmp", bufs=3) as tpool,
        tc.tile_pool(name="fin", bufs=2) as fpool,
        tc.tile_pool(name="ps", bufs=4, space="PSUM") as pspool,
        tc.tile_pool(name="psf", bufs=2, space="PSUM") as psfpool,
    ):
        iota32 = cpool.tile([128, maxns * 128], f32, tag="iota32")
        nc.gpsimd.iota(iota32[:], pattern=[[0, maxns], [1, 128]], base=0,
                       channel_multiplier=0,
                       allow_small_or_imprecise_dtypes=True)
        iota = cpool.tile([128, maxns * 128], bf16, tag="iota")
        nc.vector.tensor_copy(iota[:], iota32[:])
        ident_s = cpool.tile([128, 128], f32, tag="ident")
        nc.sync.dma_start(ident_s[:], ins["ident"][:])
        theta_s = cpool.tile([64, D], f32, tag="theta")
        nc.sync.dma_start(theta_s[:], ins["theta"][:])

        # resident denc/val tables per direction
        dvs = []
        for dirn in range(2):
            nst = lays[dirn][4]
            denc_s = wpool.tile([128, nst], bf16, tag=f"denc{dirn}")
            val_s = wpool.tile([128, nst], bf16, tag=f"val{dirn}")
            nc.sync.dma_start(denc_s[:], tabs[dirn][1][:])
            nc.sync.dma_start(val_s[:], tabs[dirn][2][:])
            dvs.append((denc_s, val_s))

        st = wpool.tile([128, GROUPS, D], f32, tag="st")
        nc.vector.memset(st[:], 0.0)

        for hop in range(0 if SKIP_SPMM else NUM_HOPS):
            if hop == 0:
                zsrc = [ins["z0a"], ins["z0b"]]
            else:
                p = (hop - 1) % 2
                zsrc = [zbuf[0][p].ap(), zbuf[1][p].ap()]
            last_hop = hop == NUM_HOPS - 1
            for dirn in range(2):
                idx_d = tabs[dirn][0]
                denc_s, val_s = dvs[dirn]
                caps = meta["caps"][dirn]
                blocks, call_off, call_slots, cell_soff, _ = lays[dirn]
                fslice = slice(dirn * D, dirn * D + D)

                xnt = None
                if not last_hop:
                    xnt = xpool.tile([128, GROUPS, D], bf16, tag="xnt")

                for c in range(NCHUNKS):
                    for bi, (g0, gc) in enumerate(blocks):
                        ns = call_slots[bi][c]
                        L = ns * 128
                        eoff = call_off[bi][c] * 128
                        soff = call_off[bi][c]
                        idx_t = spool.tile([128, (maxns * 128) // 16], i16,
                                           tag="idx")
                        nc.sync.dma_start(
                            idx_t[:, :L // 16],
                            idx_d[:, eoff // 16:(eoff + L) // 16])
                        msgs = spool.tile([128, maxns, 2 * D], bf16,
                                          tag="msgs")
                        trim = int(meta["trims"][dirn][bi, c])
                        nc.gpsimd.dma_gather(
                            out_ap=msgs[:, :ns, :],
                            in_ap=zsrc[c][MID[c]:nhalf[c], :],
                            idxs_ap=idx_t[:, :L // 16],
                            num_idxs=L,
                            num_idxs_reg=trim,
                            elem_size=2 * D,
                            single_packet=False,
                            queue_num=(bi + c) % 4,
                        )
                        S = spool2.tile([128, maxns, 128], bf16, tag="S")
                        iota_v = iota[:].rearrange(
                            "p (s c) -> p s c", c=128)[:, :ns, :]
                        nc.vector.tensor_tensor(
                            out=S[:, :ns, :], in0=iota_v,
                            in1=denc_s[:, soff:soff + ns].broadcast_to(
                                [128, ns, 128]),
                            op=EQ)
                        nc.vector.tensor_tensor(
                            out=msgs[:, :ns, fslice],
                            in0=msgs[:, :ns, fslice],
                            in1=val_s[:, soff:soff + ns].broadcast_to(
                                [128, ns, D]),
                            op=MUL)

                        psb = pspool.tile([128, gc, D], f32, tag="ps")
                        for gl in range(gc):
                            g = g0 + gl
                            cap = int(caps[g, c])
                            s0 = cell_soff[g, c] - call_off[bi][c]
                            for s in range(cap):
                                nc.tensor.matmul(
                                    psb[:, gl, :],
                                    lhsT=S[:, s0 + s, :],
                                    rhs=msgs[:, s0 + s, fslice],
                                    start=(s == 0),
                                    stop=(s == cap - 1),
                                )
                        for gl in range(gc):
                            g = g0 + gl
                            if not last_hop:
                                if c == 0:
                                    nc.scalar.copy(out=xnt[:, g, :],
                                                   in_=psb[:, gl, :])
                                else:
                                    tmpb = tpool.tile([128, gc, D], bf16,
                                                      tag="tmpb")
                                    nc.scalar.copy(out=tmpb[:, gl, :],
                                                   in_=psb[:, gl, :])
                                    nc.vector.tensor_tensor(
                                        out=xnt[:, g, :],
                                        in0=xnt[:, g, :],
                                        in1=tmpb[:, gl, :], op=ADD)
                            nc.vector.scalar_tensor_tensor(
                                out=st[:, g, :], in0=psb[:, gl, :],
                                scalar=coef[hop][dirn], in1=st[:, g, :],
                                op0=MUL, op1=ADD)
                        if c == 1 and not last_hop:
                            # bounce this block's groups, split by half
                            if g0 < GROUPS_A:
                                bh, lg0 = 0, g0
                            else:
                                bh, lg0 = 1, g0 - GROUPS_A
                            bv = bounce[bh][hop % 2].ap()[
                                lg0 * 128:(lg0 + gc) * 128, fslice].rearrange(
                                "(g p) f -> p g f", p=128)
                            nc.sync.dma_start(bv, xnt[:, g0:g0 + gc, :])
                        if (c == 1 and dirn == 1 and not last_hop
                                and g0 + gc == GROUPS_A):
                            nc.gpsimd.collective_compute(
                                "AllGather", mybir.AluOpType.bypass,
                                replica_groups=rg,
                                ins=[bounce[0][hop % 2].ap().opt()],
                                outs=[zbuf[0][hop % 2].ap().opt()],
                            )

            if not last_hop:
                nc.gpsimd.collective_compute(
                    "AllGather", mybir.AluOpType.bypass,
                    replica_groups=rg,
                    ins=[bounce[1][hop % 2].ap().opt()],
                    outs=[zbuf[1][hop % 2].ap().opt()],
                )

        # final: y_fm = sigmoid(Theta^T @ st_fm) + H_fm, feature-major
        if SKIP_FINAL:
            for g in range(GROUPS):
                yv = outs["y"][:, g * 128:(g + 1) * 128].rearrange(
                    "f p -> p f")
                nc.sync.dma_start(yv, st[:, g, :])
            return
        fchunks = [(i * 4, min(4, GROUPS - i * 4))
                   for i in range((GROUPS + 3) // 4)]
        for ci, (gs, gcnt) in enumerate(fchunks):
            width = gcnt * 128
            stfm = fpool.tile([64, width], f32, tag="stfm")
            for j in range(gcnt):
                pt = psfpool.tile([64, 128], f32, tag="pt")
                nc.tensor.transpose(pt[:], st[:, gs + j, :], ident_s[:])
                nc.scalar.copy(out=stfm[:, j * 128:(j + 1) * 128], in_=pt[:])
            zp = psfpool.tile([64, width], f32, tag="zp")
            nc.tensor.matmul(zp[:], lhsT=theta_s[:], rhs=stfm[:],
                             start=True, stop=True)
            sg = fpool.tile([64, width], f32, tag="sg")
            nc.scalar.activation(sg[:], zp[:],
                                 mybir.ActivationFunctionType.Sigmoid)
            hf = fpool.tile([64, width], f32, tag="hf")
            nc.sync.dma_start(
                hf[:], ins["hfm"][:, gs * 128:gs * 128 + width])
            yt = fpool.tile([64, width], f32, tag="yt")
            nc.vector.tensor_tensor(out=yt[:], in0=sg[:], in1=hf[:], op=ADD)
            nc.sync.dma_start(
                outs["y"][:, gs * 128:gs * 128 + width], yt[:])


def kernel(**inputs) -> np.ndarray:
    return _run(inputs, trace=False)[0]


def kernel_traced(inputs, trace_kwargs=None):
    """Returns (output, BassKernelResults) with NTFF trace if available."""
    return _run(inputs, trace=True, trace_kwargs=trace_kwargs or {})


def _run(inputs, trace=False, trace_kwargs=None):
    import concourse.bacc as bacc
    import concourse.mybir as mybir
    import concourse.tile as tile
    from concourse.bass_utils import run_bass_kernel_spmd

    in_maps, meta = prep_host(**inputs)

    nc = bacc.Bacc("TRN2", target_bir_lowering=False, debug=False,
                   num_devices=NCORES, num_swdge_queues=4)
    f32 = mybir.dt.float32
    bf16 = mybir.dt.bfloat16
    i16 = mybir.dt.int16
    tot0 = meta["lay"][0][4] * 128
    tot1 = meta["lay"][1][4] * 128

    ins = {}
    shapes = {
        "z0a": ([NA, 2 * D], bf16),
        "z0b": ([NB, 2 * D], bf16),
        "hfm": ([D, SHARD], f32),
        "theta": ([D, D], f32),
        "ident": ([128, 128], f32),
        "idx0": ([128, tot0 // 16], i16),
        "denc0": ([128, tot0 // 128], bf16),
        "val0": ([128, tot0 // 128], bf16),
        "idx1": ([128, tot1 // 16], i16),
        "denc1": ([128, tot1 // 128], bf16),
        "val1": ([128, tot1 // 128], bf16),
    }
    for k, (shape, dt) in shapes.items():
        ins[k] = nc.dram_tensor(k, shape, dt, kind="ExternalInput").ap()
    y = nc.dram_tensor("y", [D, SHARD], f32, kind="ExternalOutput")

    with tile.TileContext(nc) as tc:
        build_program(tc, ins, {"y": y.ap()}, meta)
    nc.compile()

    kw = {}
    if trace:
        kw = dict(trace=True, trace_kwargs=trace_kwargs or {})
    res = run_bass_kernel_spmd(nc, in_maps, core_ids=list(range(NCORES)),
                               **kw)
    shards = [r["y"].T for r in res.results]  # each [SHARD, 64]
    out = np.concatenate(shards, axis=0)[:N_NODES]
    return np.ascontiguousarray(out.astype(np.float32)), res


# revision 14
# speedup vs baseline: 1.2142x; 1.2142x over previous
"""CascadeGDCN (3-hop graph diffusion convolution) on 8 Trainium2 NeuronCores.

Strategy (v3):
  - Destination nodes sharded across the 8 cores (12544 rows each, padded to
    100352 total).  Edges partitioned by destination core.
  - Node features for BOTH chains packed per node: Z[v] = [X_out[v] | X_in[v]]
    as bf16, 128 wide = 256B rows (the dma_gather element granularity).
  - Gather-source node space split into two halves A (48 groups/core) and
    B (50 groups/core), renumbered so each half is contiguous across cores.
    One AllGather per half per hop; the A-half AllGather is triggered early
    (mid way through the second direction) so the next hop's A-chunk gathers
    never stall on the collective.
  - Per SpMM direction, two passes (chunk A then chunk B).  Edges bucketed
    into (128-dest-group x half) cells; signed int16 gather indices are
    relative to the half's midpoint.  Messages fetched with dma_gather;
    segment reduction on the TensorEngine as S^T @ M with a one-hot bf16 S
    built by one VectorEngine is_equal; edge values folded into the used
    64-wide half of the messages.  Per-cell slot counts are variable
    (max across cores).  Trailing padding of each gather call is marked with
    negative indices so the Q7 descriptor generator skips it.
  - Final stage: sum_term^T via PE transpose, z = Theta^T @ st_fm on PE,
    sigmoid on ScalarE, + H on VectorE, output written feature-major and
    transposed back on the host.
"""

import numpy as np
import ml_dtypes

D = 64
NCORES = 8
NUM_HOPS = 3
N_NODES = 100000
SHARD = 12544            # dest rows per core (98 groups of 128)
NODES_PAD = SHARD * NCORES   # 100352
GROUPS = SHARD // 128    # 98
GROUPS_A = 48            # groups per core in half A
SPLIT_LOCAL = GROUPS_A * 128      # 6144
NA = SPLIT_LOCAL * NCORES         # 49152 rows in half A
NB = NODES_PAD - NA               # 51200 rows in half B
MID = [NA // 2, NB // 2]          # signed-idx base offset per half
NCHUNKS = 2
GPB = 4                  # dest groups per block (per gather call)
SKIP_FINAL = False       # dev bisect flag
SKIP_SPMM = False        # dev bisect flag

BF16 = ml_dtypes.bfloat16


def _softmax(x):
    e = np.exp(x - x.max())
    return e / e.sum()


def _new_id(v):
    """Renumber original node id -> packed [A | B] gather-space id."""
    m = v // SHARD
    loc = v % SHARD
    return np.where(loc < SPLIT_LOCAL,
                    m * SPLIT_LOCAL + loc,
                    NA + m * (SHARD - SPLIT_LOCAL) + (loc - SPLIT_LOCAL))


def _blocks():
    out = []
    g = 0
    while g < GROUPS:
        out.append((g, min(GPB, GROUPS - g)))
        g += GPB
    return out


def _layout(caps):
    """Edge-stream layout from per-cell slot counts caps[g, c].

    Stream order: [chunk c][block][group_in_block cells consecutively]
    (pass-major so each pass's calls are contiguous in the stream).
    """
    blocks = _blocks()
    call_off = [[0] * NCHUNKS for _ in blocks]
    call_slots = [[0] * NCHUNKS for _ in blocks]
    cell_soff = np.zeros((GROUPS, NCHUNKS), dtype=np.int64)
    pos = 0
    for c in range(NCHUNKS):
        for bi, (g0, gc) in enumerate(blocks):
            call_off[bi][c] = pos
            n = 0
            for gl in range(gc):
                cell_soff[g0 + gl, c] = pos + n
                n += int(caps[g0 + gl, c])
            pos += n
            call_slots[bi][c] = n
    return blocks, call_off, call_slots, cell_soff, pos


def _direction_caps(dest, src, shard):
    """caps[g, c] = max over cores of ceil(cell_count/128), min 1."""
    core = dest // shard
    c_of = ((src % shard) >= SPLIT_LOCAL).astype(np.int64)
    caps = np.zeros((GROUPS, NCHUNKS), dtype=np.int64)
    for m in range(NCORES):
        sel = core == m
        d_loc = dest[sel] - m * shard
        g = d_loc >> 7
        counts = np.bincount(g * NCHUNKS + c_of[sel],
                             minlength=GROUPS * NCHUNKS)
        caps = np.maximum(caps, (counts.reshape(GROUPS, NCHUNKS) + 127) // 128)
    return np.maximum(caps, 1)


def _prep_direction(dest, src, val, shard, caps, lay):
    """Per-core gather/S tables for one SpMM direction."""
    blocks, call_off, call_slots, cell_soff, tot_slots = lay
    tot = tot_slots * 128
    core = dest // shard
    src_new = _new_id(src)
    c_of = ((src % shard) >= SPLIT_LOCAL).astype(np.int64)
    base = np.where(c_of == 0, MID[0], NA + MID[1])
    idx_all = src_new - base                      # signed, fits int16
    out = []
    for m in range(NCORES):
        sel = core == m
        d_loc = (dest[sel] - m * shard).astype(np.int64)
        idxv = idx_all[sel].astype(np.int64)
        v = val[sel].astype(np.float32)
        g = d_loc >> 7
        cell = g * NCHUNKS + c_of[sel]
        order = np.lexsort((idxv, cell))
        cell_s = cell[order]
        counts = np.bincount(cell_s, minlength=GROUPS * NCHUNKS)
        starts = np.zeros(GROUPS * NCHUNKS, dtype=np.int64)
        starts[1:] = np.cumsum(counts)[:-1]
        rank = np.arange(cell_s.size) - starts[cell_s]
        pos = cell_soff.reshape(-1)[cell_s] * 128 + rank

        idx_st = np.zeros(tot, dtype=np.int16)
        denc_st = np.full(tot, -1.0, dtype=np.float32)
        val_st = np.zeros(tot, dtype=np.float32)
        idx_st[pos] = idxv[order].astype(np.int16)
        denc_st[pos] = (d_loc[order] & 127).astype(np.float32)
        val_st[pos] = v[order]

        # record this core's last-real-edge position per call (for the
        # uniform trailing-trim computed across cores by the caller)
        counts2 = counts.reshape(GROUPS, NCHUNKS)
        last_real = np.zeros((len(blocks), NCHUNKS), dtype=np.int64)
        for bi, (g0, gc) in enumerate(blocks):
            for c in range(NCHUNKS):
                gl_last = g0 + gc - 1
                last_real[bi, c] = (cell_soff[gl_last, c] * 128
                                    + counts2[gl_last, c])

        out.append({"idx_st": idx_st, "denc_st": denc_st, "val_st": val_st,
                    "last_real": last_real})

    # uniform trailing-trim point per call (max across cores), so that every
    # core's Q7 trims to the same count and it can be baked into num_idxs_reg
    trims = np.zeros((len(blocks), NCHUNKS), dtype=np.int64)
    for bi, (g0, gc) in enumerate(blocks):
        for c in range(NCHUNKS):
            call_end = (call_off[bi][c] + call_slots[bi][c]) * 128
            # no trailing trim: skipped descriptors would leave stale SBUF
            # rows in msgs, and S=0 x stale-NaN = NaN in PSUM
            t = call_end
            trims[bi, c] = t - call_off[bi][c] * 128   # call-relative count

    tables = []
    for o in out:
        idx_tbl = np.tile(
            np.ascontiguousarray(o["idx_st"].reshape(-1, 16).T), (8, 1))
        denc_tbl = np.ascontiguousarray(
            o["denc_st"].reshape(-1, 128).T.astype(BF16))
        val_tbl = np.ascontiguousarray(
            o["val_st"].reshape(-1, 128).T.astype(BF16))
        tables.append({"idx": idx_tbl, "denc": denc_tbl, "val": val_tbl})
    return tables, trims


def prep_host(H_l, edge_row, edge_col, edge_val, out_degree, in_degree,
              hop_attention, theta_out, theta_in, Theta):
    """Host-side preprocessing: per-core input maps + meta for the builder."""
    H = np.asarray(H_l, dtype=np.float32)
    er = np.asarray(edge_row, dtype=np.int64)
    ec = np.asarray(edge_col, dtype=np.int64)
    ev = np.asarray(edge_val, dtype=np.float32)
    od = np.asarray(out_degree, dtype=np.float32)
    idg = np.asarray(in_degree, dtype=np.float32)

    alpha = _softmax(np.asarray(hop_attention, dtype=np.float64))
    th_o = np.asarray(theta_out, dtype=np.float64)
    th_i = np.asarray(theta_in, dtype=np.float64)
    coef = [(float(alpha[k] * th_o[k]), float(alpha[k] * th_i[k]))
            for k in range(len(alpha))]

    # dir 0 ("out" chain): dest=row, src=col; dir 1: transposed
    caps0 = _direction_caps(er, ec, SHARD)
    caps1 = _direction_caps(ec, er, SHARD)
    lay0 = _layout(caps0)
    lay1 = _layout(caps1)
    t0, trims0 = _prep_direction(er, ec, ev, SHARD, caps0, lay0)
    t1, trims1 = _prep_direction(ec, er, ev, SHARD, caps1, lay1)

    # packed hop-0 features Z = [D_out H | D_in H] bf16, in renumbered order
    zfull = np.zeros((NODES_PAD, 2 * D), dtype=BF16)
    perm = _new_id(np.arange(NODES_PAD))
    zsrc = np.zeros((NODES_PAD, 2 * D), dtype=BF16)
    zsrc[:N_NODES, :D] = (np.maximum(od, 1e-8)[:, None] * H).astype(BF16)
    zsrc[:N_NODES, D:] = (np.maximum(idg, 1e-8)[:, None] * H).astype(BF16)
    zfull[perm] = zsrc
    z0a = np.ascontiguousarray(zfull[:NA])
    z0b = np.ascontiguousarray(zfull[NA:])

    hpad = np.zeros((NODES_PAD, D), dtype=np.float32)
    hpad[:N_NODES] = H
    ident = np.eye(128, dtype=np.float32)
    theta = np.ascontiguousarray(np.asarray(Theta, dtype=np.float32))

    in_maps = []
    for m in range(NCORES):
        in_maps.append({
            "z0a": z0a, "z0b": z0b,
            "hfm": np.ascontiguousarray(hpad[m * SHARD:(m + 1) * SHARD].T),
            "theta": theta,
            "ident": ident,
            "idx0": t0[m]["idx"], "denc0": t0[m]["denc"], "val0": t0[m]["val"],
            "idx1": t1[m]["idx"], "denc1": t1[m]["denc"], "val1": t1[m]["val"],
        })
    meta = {"coef": coef, "caps": [caps0, caps1], "lay": [lay0, lay1],
            "trims": [trims0, trims1]}
    return in_maps, meta


def build_program(tc, ins, outs, meta):
    """Emit the full SPMD program into TileContext tc."""
    import concourse.mybir as mybir

    nc = tc.nc
    f32 = mybir.dt.float32
    bf16 = mybir.dt.bfloat16
    i16 = mybir.dt.int16
    EQ, MUL, ADD = (mybir.AluOpType.is_equal, mybir.AluOpType.mult,
                    mybir.AluOpType.add)

    coef = meta["coef"]
    lays = meta["lay"]
    maxns = max(max(max(cs) for cs in lay[2]) for lay in lays)

    # internal DRAM: per-half bounce shards + ping-pong per-half Z buffers
    shp = [SPLIT_LOCAL, SHARD - SPLIT_LOCAL]
    nhalf = [NA, NB]
    bounce = [[nc.dram_tensor(f"bounce{h}_{p}", [shp[h], 2 * D], bf16,
                              kind="Internal") for p in range(2)]
              for h in range(2)]
    zbuf = [[nc.dram_tensor(f"zbuf{h}_{p}", [nhalf[h], 2 * D], bf16,
                            kind="Internal", addr_space="Shared")
             for p in range(2)] for h in range(2)]

    tabs = [
        (ins["idx0"], ins["denc0"], ins["val0"]),
        (ins["idx1"], ins["denc1"], ins["val1"]),
    ]
    rg = [list(range(NCORES))]

    with (
        tc.tile_pool(name="const", bufs=1) as cpool,
        tc.tile_pool(name="work", bufs=1) as wpool,
        tc.tile_pool(name="stream", bufs=5) as spool,
        tc.tile_pool(name="spool2", bufs=5) as spool2,
        tc.tile_pool(name="xn", bufs=2) as xpool,
        tc.tile_pool(name="xtmp", bufs=3) as tpool,
        tc.tile_pool(name="fin", bufs=2) as fpool,
        tc.tile_pool(name="ps", bufs=4, space="PSUM") as pspool,
        tc.tile_pool(name="psf", bufs=2, space="PSUM") as psfpool,
    ):
        iota32 = cpool.tile([128, maxns * 128], f32, tag="iota32")
        nc.gpsimd.iota(iota32[:], pattern=[[0, maxns], [1, 128]], base=0,
                       channel_multiplier=0,
                       allow_small_or_imprecise_dtypes=True)
        iota = cpool.tile([128, maxns * 128], bf16, tag="iota")
        nc.vector.tensor_copy(iota[:], iota32[:])
        ident_s = cpool.tile([128, 128], f32, tag="ident")
        nc.sync.dma_start(ident_s[:], ins["ident"][:])
        theta_s = cpool.tile([64, D], f32, tag="theta")
        nc.sync.dma_start(theta_s[:], ins["theta"][:])

        # resident denc/val tables per direction
        dvs = []
        for dirn in range(2):
            nst = lays[dirn][4]
            denc_s = wpool.tile([128, nst], bf16, tag=f"denc{dirn}")
            val_s = wpool.tile([128, nst], bf16, tag=f"val{dirn}")
            nc.sync.dma_start(denc_s[:], tabs[dirn][1][:])
            nc.sync.dma_start(val_s[:], tabs[dirn][2][:])
            dvs.append((denc_s, val_s))

        st = wpool.tile([128, GROUPS, D], f32, tag="st")
        nc.vector.memset(st[:], 0.0)

        for hop in range(0 if SKIP_SPMM else NUM_HOPS):
            if hop == 0:
                zsrc = [ins["z0a"], ins["z0b"]]
            else:
                p = (hop - 1) % 2
                zsrc = [zbuf[0][p].ap(), zbuf[1][p].ap()]
            last_hop = hop == NUM_HOPS - 1
            for dirn in range(2):
                idx_d = tabs[dirn][0]
                denc_s, val_s = dvs[dirn]
                caps = meta["caps"][dirn]
                blocks, call_off, call_slots, cell_soff, _ = lays[dirn]
                fslice = slice(dirn * D, dirn * D + D)

                xnt = None
                if not last_hop:
                    xnt = xpool.tile([128, GROUPS, D], bf16, tag="xnt")

                for c in range(NCHUNKS):
                    for bi, (g0, gc) in enumerate(blocks):
                        ns = call_slots[bi][c]
                        L = ns * 128
                        eoff = call_off[bi][c] * 128
                        soff = call_off[bi][c]
                        idx_t = spool.tile([128, (maxns * 128) // 16], i16,
                                           tag="idx")
                        nc.sync.dma_start(
                            idx_t[:, :L // 16],
                            idx_d[:, eoff // 16:(eoff + L) // 16])
                        msgs = spool.tile([128, maxns, 2 * D], bf16,
                                          tag="msgs")
                        trim = int(meta["trims"][dirn][bi, c])
                        nc.gpsimd.dma_gather(
                            out_ap=msgs[:, :ns, :],
                            in_ap=zsrc[c][MID[c]:nhalf[c], :],
                            idxs_ap=idx_t[:, :L // 16],
                            num_idxs=L,
                            num_idxs_reg=trim,
                            elem_size=2 * D,
                            single_packet=False,
                            queue_num=(bi + c) % 4,
                        )
                        S = spool2.tile([128, maxns, 128], bf16, tag="S")
                        iota_v = iota[:].rearrange(
                            "p (s c) -> p s c", c=128)[:, :ns, :]
                        nc.vector.tensor_tensor(
                            out=S[:, :ns, :], in0=iota_v,
                            in1=denc_s[:, soff:soff + ns].broadcast_to(
                                [128, ns, 128]),
                            op=EQ)
                        nc.vector.tensor_tensor(
                            out=msgs[:, :ns, fslice],
                            in0=msgs[:, :ns, fslice],
                            in1=val_s[:, soff:soff + ns].broadcast_to(
                                [128, ns, D]),
                            op=MUL)

                        psb = pspool.tile([128, gc, D], f32, tag="ps")
                        for gl in range(gc):
                            g = g0 + gl
                            cap = int(caps[g, c])
                            s0 = cell_soff[g, c] - call_off[bi][c]
                            for s in range(cap):
                                nc.tensor.matmul(
                                    psb[:, gl, :],
                                    lhsT=S[:, s0 + s, :],
                                    rhs=msgs[:, s0 + s, fslice],
                                    start=(s == 0),
                                    stop=(s == cap - 1),
                                )
                        if not last_hop:
                            if c == 0:
                                nc.scalar.copy(out=xnt[:, g0:g0 + gc, :],
                                               in_=psb[:])
                            else:
                                tmpb = tpool.tile([128, gc, D], bf16,
                                                  tag="tmpb")
                                nc.scalar.copy(out=tmpb[:], in_=psb[:])
                                nc.vector.tensor_tensor(
                                    out=xnt[:, g0:g0 + gc, :],
                                    in0=xnt[:, g0:g0 + gc, :],
                                    in1=tmpb[:], op=ADD)
                        nc.vector.scalar_tensor_tensor(
                            out=st[:, g0:g0 + gc, :], in0=psb[:],
                            scalar=coef[hop][dirn],
                            in1=st[:, g0:g0 + gc, :],
                            op0=MUL, op1=ADD)
                        if c == 1 and not last_hop:
                            # bounce this block's groups, split by half
                            if g0 < GROUPS_A:
                                bh, lg0 = 0, g0
                            else:
                                bh, lg0 = 1, g0 - GROUPS_A
                            bv = bounce[bh][hop % 2].ap()[
                                lg0 * 128:(lg0 + gc) * 128, fslice].rearrange(
                                "(g p) f -> p g f", p=128)
                            nc.sync.dma_start(bv, xnt[:, g0:g0 + gc, :])
                        if (c == 1 and dirn == 1 and not last_hop
                                and g0 + gc == GROUPS_A):
                            nc.gpsimd.collective_compute(
                                "AllGather", mybir.AluOpType.bypass,
                                replica_groups=rg,
                                ins=[bounce[0][hop % 2].ap().opt()],
                                outs=[zbuf[0][hop % 2].ap().opt()],
                            )

            if not last_hop:
                nc.gpsimd.collective_compute(
                    "AllGather", mybir.AluOpType.bypass,
                    replica_groups=rg,
                    ins=[bounce[1][hop % 2].ap().opt()],
                    outs=[zbuf[1][hop % 2].ap().opt()],
                )

        # final: y_fm = sigmoid(Theta^T @ st_fm) + H_fm, feature-major
        if SKIP_FINAL:
            for g in range(GROUPS):
                yv = outs["y"][:, g * 128:(g + 1) * 128].rearrange(
                    "f p -> p f")
                nc.sync.dma_start(yv, st[:, g, :])
            return
        fchunks = [(i * 4, min(4, GROUPS - i * 4))
                   for i in range((GROUPS + 3) // 4)]
        for ci, (gs, gcnt) in enumerate(fchunks):
            width = gcnt * 128
            stfm = fpool.tile([64, width], f32, tag="stfm")
            for j in range(gcnt):
                pt = psfpool.tile([64, 128], f32, tag="pt")
                nc.tensor.transpose(pt[:], st[:, gs + j, :], ident_s[:])
                nc.scalar.copy(out=stfm[:, j * 128:(j + 1) * 128], in_=pt[:])
            zp = psfpool.tile([64, width], f32, tag="zp")
            nc.tensor.matmul(zp[:], lhsT=theta_s[:], rhs=stfm[:],
                             start=True, stop=True)
            sg = fpool.tile([64, width], f32, tag="sg")
            nc.scalar.activation(sg[:], zp[:],
                                 mybir.ActivationFunctionType.Sigmoid)
            hf = fpool.tile([64, width], f32, tag="hf")
            nc.sync.dma_start(
                hf[:], ins["hfm"][:, gs * 128:gs * 128 + width])
            yt = fpool.tile([64, width], f32, tag="yt")
            nc.vector.tensor_tensor(out=yt[:], in0=sg[:], in1=hf[:], op=ADD)
            nc.sync.dma_start(
                outs["y"][:, gs * 128:gs * 128 + width], yt[:])


def kernel(**inputs) -> np.ndarray:
    return _run(inputs, trace=False)[0]


def kernel_traced(inputs, trace_kwargs=None):
    """Returns (output, BassKernelResults) with NTFF trace if available."""
    return _run(inputs, trace=True, trace_kwargs=trace_kwargs or {})


def _run(inputs, trace=False, trace_kwargs=None):
    import concourse.bacc as bacc
    import concourse.mybir as mybir
    import concourse.tile as tile
    from concourse.bass_utils import run_bass_kernel_spmd

    in_maps, meta = prep_host(**inputs)

    nc = bacc.Bacc("TRN2", target_bir_lowering=False, debug=False,
                   num_devices=NCORES, num_swdge_queues=4)
    f32 = mybir.dt.float32
    bf16 = mybir.dt.bfloat16
    i16 = mybir.dt.int16
    tot0 = meta["lay"][0][4] * 128
    tot1 = meta["lay"][1][4] * 128

    ins = {}
    shapes = {
        "z0a": ([NA, 2 * D], bf16),
        "z0b": ([NB, 2 * D], bf16),
        "hfm": ([D, SHARD], f32),
        "theta": ([D, D], f32),
        "ident": ([128, 128], f32),
        "idx0": ([128, tot0 // 16], i16),
        "denc0": ([128, tot0 // 128], bf16),
        "val0": ([128, tot0 // 128], bf16),
        "idx1": ([128, tot1 // 16], i16),
        "denc1": ([128, tot1 // 128], bf16),
        "val1": ([128, tot1 // 128], bf16),
    }
    for k, (shape, dt) in shapes.items():
        ins[k] = nc.dram_tensor(k, shape, dt, kind="ExternalInput").ap()
    y = nc.dram_tensor("y", [D, SHARD], f32, kind="ExternalOutput")

    with tile.TileContext(nc) as tc:
        build_program(tc, ins, {"y": y.ap()}, meta)
    nc.compile()

    kw = {}
    if trace:
        kw = dict(trace=True, trace_kwargs=trace_kwargs or {})
    res = run_bass_kernel_spmd(nc, in_maps, core_ids=list(range(NCORES)),
                               **kw)
    shards = [r["y"].T for r in res.results]  # each [SHARD, 64]
    out = np.concatenate(shards, axis=0)[:N_NODES]
    return np.ascontiguousarray(out.astype(np.float32)), res
